# revision 20
# baseline (speedup 1.0000x reference)
"""Trainium2 Bass kernel for nn_LoopModel2: out = x + sum(range(y)).

The loop `for i in range(y): x = x + i` collapses to a single elementwise
add of the constant y*(y-1)/2 (2016.0 for y=64). That makes the kernel a
pure HBM-streaming problem: DMA tiles of x into SBUF, add the constant on
the vector engine, DMA back out. x (8192, 8192) f32 is sharded row-wise
across the 8 NeuronCores; no communication is needed.

Per-core structure (shard = 1024 x 8192 f32 = 32 MiB, seen as 8 tiles of
[128, 8192] = 4 MiB):
  - loads ride the SP HWDGE ring (nc.sync), stores the ACT ring
    (nc.scalar). With both queue rows feeding the 16 SDMA engines the
    steady-state DMA rate sits at ~433 GB/s, the SBUF AXI fabric ceiling
    (435 GB/s); a single ring saturates at ~340 GB/s.
  - bufs=6 SBUF slots let loads run well ahead and absorb DMA jitter.
  - built on bacc.Bacc: its finalize() runs generate_event_semaphores,
    which splits multi-semaphore waits off DMA/compute instructions
    (walrus codegen rejects >1 inline sync wait per instruction).

Measured on trn2 (8 cores, SPMD): ~168 us NEFF exec vs a ~155 us fabric
roofline (64 MiB of DMA per core at 435 GB/s).
"""

import os

import numpy as np

import concourse.bacc as bacc
import concourse.mybir as mybir
from concourse.tile import TileContext
from concourse.bass_utils import run_bass_kernel_spmd

N_CORES = 8
ROWS, COLS = 8192, 8192
SHARD_ROWS = ROWS // N_CORES  # 1024 rows per core

# Tiling of one core's 32 MiB shard: NT tiles of [P, F] f32.
P = 128
F = 8192
NT = (SHARD_ROWS * COLS) // (P * F)  # 8
BUFS = 6

# Filled in by the last traced run (the local test harness reads these).
LAST_EXEC_NS = None
LAST_RESULTS = None

_cache = {}


def _build(const: float):
    nc = bacc.Bacc()
    x_in = nc.dram_tensor("x", [NT, P, F], mybir.dt.float32, kind="ExternalInput")
    out = nc.dram_tensor("out", [NT, P, F], mybir.dt.float32, kind="ExternalOutput")

    with TileContext(nc) as tc:
        with tc.tile_pool(name="io", bufs=BUFS) as pool:
            for i in range(NT):
                t = pool.tile([P, F], mybir.dt.float32)
                nc.sync.dma_start(out=t[:], in_=x_in[i])
                nc.vector.tensor_scalar_add(t[:], t[:], const)
                nc.scalar.dma_start(out=out[i], in_=t[:])
    nc.finalize()
    return nc


def kernel(x, y) -> np.ndarray:
    global LAST_EXEC_NS, LAST_RESULTS
    y = int(y)
    const = float(y * (y - 1) // 2)

    if const not in _cache:
        _cache[const] = _build(const)
    nc = _cache[const]

    x_np = np.asarray(x, dtype=np.float32)
    in_maps = [
        {"x": x_np[c * SHARD_ROWS:(c + 1) * SHARD_ROWS].reshape(NT, P, F)}
        for c in range(N_CORES)
    ]
    trace = bool(os.environ.get("KERNEL_TRACE"))
    res = run_bass_kernel_spmd(nc, in_maps, list(range(N_CORES)), trace=trace)
    LAST_EXEC_NS = res.exec_time_ns
    LAST_RESULTS = res

    out = np.empty((ROWS, COLS), dtype=np.float32)
    for c in range(N_CORES):
        out[c * SHARD_ROWS:(c + 1) * SHARD_ROWS] = (
            res.results[c]["out"].reshape(SHARD_ROWS, COLS)
        )
    return out


# revision 22
# speedup vs baseline: 1.1525x; 1.1525x over previous
"""Trainium2 Bass kernel for nn_LoopModel2: out = x + sum(range(y)).

The loop `for i in range(y): x = x + i` collapses to a single elementwise
add of the constant y*(y-1)/2 (2016.0 for y=64). That makes the kernel a
pure HBM-streaming problem: DMA tiles of x into SBUF, add the constant on
the vector engine, DMA back out. x (8192, 8192) f32 is sharded row-wise
across the 8 NeuronCores; no communication is needed.

Per-core structure (shard = 1024 x 8192 f32 = 32 MiB, seen as 8 tiles of
[128, 8192] = 4 MiB):
  - loads ride the SP HWDGE ring (nc.sync), stores the ACT ring
    (nc.scalar). With both queue rows feeding the 16 SDMA engines the
    steady-state DMA rate sits at ~433 GB/s, the SBUF AXI fabric ceiling
    (435 GB/s); a single ring saturates at ~340 GB/s.
  - bufs=6 SBUF slots let loads run well ahead and absorb DMA jitter.
  - built on bacc.Bacc: its finalize() runs generate_event_semaphores,
    which splits multi-semaphore waits off DMA/compute instructions
    (walrus codegen rejects >1 inline sync wait per instruction).

Measured on trn2 (8 cores, SPMD): ~168 us NEFF exec vs a ~155 us fabric
roofline (64 MiB of DMA per core at 435 GB/s).
"""

import os

import numpy as np

import concourse.bacc as bacc
import concourse.mybir as mybir
from concourse.tile import TileContext
from concourse.bass_utils import run_bass_kernel_spmd

N_CORES = 8
ROWS, COLS = 8192, 8192
SHARD_ROWS = ROWS // N_CORES  # 1024 rows per core

# Tiling of one core's 32 MiB shard: NT tiles of [P, F] f32.
P = 128
F = 8192
NT = (SHARD_ROWS * COLS) // (P * F)  # 8
BUFS = 6
# KPRIME=1: issue load 1 on the ACT ring so both HWDGE rings pull from
# t=0 (the SP ring alone caps at ~340 GB/s during the ramp).
PRIME = bool(int(os.environ.get("KPRIME", "0")))

# Filled in by the last traced run (the local test harness reads these).
LAST_EXEC_NS = None
LAST_RESULTS = None

_cache = {}


def _build(const: float):
    nc = bacc.Bacc()
    x_in = nc.dram_tensor("x", [NT, P, F], mybir.dt.float32, kind="ExternalInput")
    out = nc.dram_tensor("out", [NT, P, F], mybir.dt.float32, kind="ExternalOutput")

    with TileContext(nc) as tc:
        with tc.tile_pool(name="io", bufs=BUFS) as pool:
            for i in range(NT):
                t = pool.tile([P, F], mybir.dt.float32)
                load_eng = nc.scalar if (PRIME and i == 1) else nc.sync
                load_eng.dma_start(out=t[:], in_=x_in[i])
                nc.vector.tensor_scalar_add(t[:], t[:], const)
                nc.scalar.dma_start(out=out[i], in_=t[:])
    nc.finalize()
    return nc


def kernel(x, y) -> np.ndarray:
    global LAST_EXEC_NS, LAST_RESULTS
    y = int(y)
    const = float(y * (y - 1) // 2)

    if const not in _cache:
        _cache[const] = _build(const)
    nc = _cache[const]

    x_np = np.asarray(x, dtype=np.float32)
    in_maps = [
        {"x": x_np[c * SHARD_ROWS:(c + 1) * SHARD_ROWS].reshape(NT, P, F)}
        for c in range(N_CORES)
    ]
    trace = bool(os.environ.get("KERNEL_TRACE"))
    res = run_bass_kernel_spmd(nc, in_maps, list(range(N_CORES)), trace=trace)
    LAST_EXEC_NS = res.exec_time_ns
    LAST_RESULTS = res

    out = np.empty((ROWS, COLS), dtype=np.float32)
    for c in range(N_CORES):
        out[c * SHARD_ROWS:(c + 1) * SHARD_ROWS] = (
            res.results[c]["out"].reshape(SHARD_ROWS, COLS)
        )
    return out
